# revision 10
# baseline (speedup 1.0000x reference)
"""DCT-attention Trainium2 kernel (8 NeuronCores, data-parallel over batch).

Reference math (per b, h):
    Qd = dct @ (Q*s);  Kd = dct @ (K*s*mask);  Vd = dct @ (V*mask)   # [M,D]
    E  = Qd @ Kd^T;  P = softmax(E, axis=-1);  ctx = P @ Vd          # [M,D]
    x  = dct^T @ ctx                                                 # [N,D]
with B,H,N,D = 8,12,2048,64, M = 256, s = D**-0.25.

Sharding: batch b -> core b (8 cores). Host folds scale into Q/K and mask into
K/V, transposes to [N, H*D], bf16-casts; matmuls run bf16 -> fp32 PSUM; output
returns bf16 and is cast to f32 on the host.

DCT parity symmetry: dct[k, N-1-i] = (-1)^k dct[k, i].  The host therefore
uploads X folded as [A; B] with A = X[:N/2] + reverse(X[N/2:]),
B = X[:N/2] - reverse(X[N/2:]), and the M axis globally reordered to
[even k | odd k] (dctT columns, dct rows, and everything downstream use the
same order; softmax/ctx are permutation-invariant).  Every projection then
contracts over N/2 instead of N — half the matmul work.

Schedule per core (input DMA on the sync HWDGE queue; the PE chases the
arrival stream; out DMA alternates sync / gpsimd queues):
  DMA:  dctT | q (2+2+4+4+4 chunks) | k | v (4-chunk quarters) | dct
  PE:   Q-proj A/B -> K-proj A -> K-proj B + even-k energy pairs ->
        V-proj A + odd-k energy pairs -> V-proj B + phase-B kb0
        pre-accumulation -> kb1 + normalize -> inverse-DCT sweep
        (full-width rows, out piece DMA every 2 row-blocks).
Energies pack 2 heads (64-row tiles) per PSUM bank; one 512-wide exp per
pair.  Phase-B partial sums pack 6 heads per PSUM bank so all 24 (h, mb)
accumulations stay open across the V stream.
"""

import numpy as np
import ml_dtypes

B, H, N, D = 8, 12, 2048, 64
M = 256
HD = H * D          # 768
NH = N // 2         # 1024 folded length
NCH = NH // 128     # 8 folded chunks per parity phase
MB = M // 128       # 2 m-blocks (even ks | odd ks)
HP = H // 2         # 6 head-pairs
FSPLIT = 2          # HD split for <=512-wide psum
FW = HD // FSPLIT   # 384
VW = D + 1          # 65: Vd columns + ones column
NB = NH // 128      # 8 output row-blocks per half
PCH = 2             # row-blocks per out piece

_BF16 = ml_dtypes.bfloat16
_CACHE = {}

# q is split finer at the head of the stream so the PE starts sooner.
Q_PIECES = [2, 2, 4, 4, 4]
KV_PIECES = [4, 4, 4, 4]

import os
PACK_PC = os.environ.get("KPACK", "1") == "1"      # packed phase-B psum
PAIR_E = os.environ.get("KPAIRE", "1") == "1"      # paired energy psum


def build_nc():
    import concourse.bacc as bacc
    import concourse.mybir as mybir
    import concourse.tile as tile
    from contextlib import ExitStack

    BF = mybir.dt.bfloat16
    F32 = mybir.dt.float32
    EXP = mybir.ActivationFunctionType.Exp
    COPY = mybir.ActivationFunctionType.Copy

    nc = bacc.Bacc()
    q_d = nc.declare_dram_parameter("q", [N, HD], BF, isOutput=False)
    k_d = nc.declare_dram_parameter("k", [N, HD], BF, isOutput=False)
    v_d = nc.declare_dram_parameter("v", [N, HD], BF, isOutput=False)
    # dctT: [n' < N/2, m] with columns [even k | odd k]
    dctT_d = nc.declare_dram_parameter("dctT", [NH, M], BF, isOutput=False)
    # dct: [M, N/2] rows reordered [even k | odd k]; the upper output half is
    # reconstructed from the same columns via x[N-1-i] = even - odd (host
    # un-reverses those rows).
    dct_d = nc.declare_dram_parameter("dct", [M, NH], BF, isOutput=False)
    out_d = nc.declare_dram_parameter("out", [N, HD], BF, isOutput=True)

    q_r = q_d.ap().rearrange("(c p) f -> p c f", p=128)
    k_r = k_d.ap().rearrange("(c p) f -> p c f", p=128)
    v_r = v_d.ap().rearrange("(c p) f -> p c f", p=128)
    dctT_r = dctT_d.ap().rearrange("(c p) m -> p c m", p=128)
    dct_r = dct_d.ap().rearrange("(b p) n -> p b n", p=128)
    out_r = out_d.ap().rearrange("(c p) f -> p c f", p=128)

    with ExitStack() as ctx:
        tc = ctx.enter_context(tile.TileContext(nc))
        consts = ctx.enter_context(tc.tile_pool(name="consts", bufs=1))
        xin = ctx.enter_context(tc.tile_pool(name="xin", bufs=1))
        proj = ctx.enter_context(tc.tile_pool(name="proj", bufs=1))
        pbuf = ctx.enter_context(tc.tile_pool(name="pbuf", bufs=1))
        rbuf = ctx.enter_context(tc.tile_pool(name="rbuf", bufs=4))
        ostage = ctx.enter_context(tc.tile_pool(name="ostage", bufs=4))
        psA = ctx.enter_context(tc.tile_pool(name="psA", bufs=6, space="PSUM"))
        psE = ctx.enter_context(tc.tile_pool(name="psE", bufs=2, space="PSUM"))

        # ---- DMA stream (sync queue; order == consumption order) ----
        dctT_sb = consts.tile([128, NCH, M], BF)       # [n'-part, chunk, m]
        nc.sync.dma_start(dctT_sb[:], dctT_r)

        def stream(name, src_r, pieces):
            chunk_map = []
            c0 = 0
            for pi, nch in enumerate(pieces):
                t = xin.tile([128, nch, HD], BF, tag=f"{name}{pi}")
                nc.sync.dma_start(t[:], src_r[:, c0:c0 + nch, :])
                for j in range(nch):
                    chunk_map.append((t, j))
                c0 += nch
            return chunk_map

        q_t = stream("q", q_r, Q_PIECES)   # chunks 0..7 = A-fold, 8..15 = B
        k_t = stream("k", k_r, KV_PIECES)
        v_t = stream("v", v_r, KV_PIECES)

        dct_sb = consts.tile([128, MB, NH], BF)        # [m-part, m-block, n']
        nc.sync.dma_start(dct_sb[:], dct_r)

        # ---- persistent intermediates ----
        qdT_sb = proj.tile([128, HP, M], BF, tag="qdT")   # [2-head d, pair, m]
        kdT_sb = proj.tile([128, HP, M], BF, tag="kdT")
        vd_sb = proj.tile([128, MB, H, VW], BF, tag="vd")  # [k-part, kb, h, d+1]
        ctx_sb = proj.tile([128, MB, HD], BF, tag="ctx")   # [m-part, mb, h*d]
        ctxn_sb = proj.tile([128, HD], BF, tag="ctxn")     # -ctx odd block
        nc.vector.memset(vd_sb[:, :, :, D:VW], 1.0)
        ebias = consts.tile([128, 1], F32)
        nc.vector.memset(ebias[:], -4.0)

        def xc(cm, c):  # folded chunk c (0..15), [128, HD]
            t, j = cm[c]
            return t[:, j, :]

        # ---- Q/K projections: per parity phase, chunk-major over 6 groups --
        def proj_phase(cm, dst_sb, par):
            # par 0: A chunks 0..7 against even-k dctT cols -> dst cols 0:128
            # par 1: B chunks 8..15 against odd-k cols     -> dst cols 128:256
            groups = [
                psA.tile([128, 128], F32, tag="A", name=f"pg{par}{hp}")
                for hp in range(HP)
            ]
            for c in range(NCH):
                for hp in range(HP):
                    nc.tensor.matmul(
                        groups[hp][:],
                        lhsT=xc(cm, par * NCH + c)[:, hp * 128:(hp + 1) * 128],
                        rhs=dctT_sb[:, c, par * 128:(par + 1) * 128],
                        start=(c == 0),
                        stop=(c == NCH - 1),
                    )
            for hp in range(HP):
                nc.vector.tensor_copy(
                    dst_sb[:, hp, par * 128:(par + 1) * 128], groups[hp][:]
                )

        proj_phase(q_t, qdT_sb, 0)
        proj_phase(q_t, qdT_sb, 1)
        proj_phase(k_t, kdT_sb, 0)

        # ---- energy pairs: 2 heads of one head-pair share a PSUM bank ----
        p_tiles = [None] * HP   # [128, MB, 2, M] bf16 per head-pair

        def emit_energy_pair(hp, kb):
            if p_tiles[hp] is None:
                p_tiles[hp] = pbuf.tile(
                    [128, MB, 2, M], BF, tag=f"p{hp}", name=f"p{hp}"
                )
            if PAIR_E:
                pe = psE.tile([128, 2, M], F32, tag="E", name=f"e{hp}{kb}")
                for j in range(2):
                    nc.tensor.matmul(
                        pe[:, j, :],
                        lhsT=kdT_sb[64 * j:64 * j + 64, hp, kb * 128:(kb + 1) * 128],
                        rhs=qdT_sb[64 * j:64 * j + 64, hp, :],
                        start=(j == 0),
                        stop=(j == 1),
                    )
                # P^T[k-block, m] = exp(E^T - 4); the -4 cancels in the
                # normalization and guards exp overflow for outlier logits.
                nc.scalar.activation(
                    p_tiles[hp][:, kb, :, :], pe[:], EXP, bias=ebias[:]
                )
            else:
                for j in range(2):
                    pe = psE.tile([128, M], F32, tag="E", name=f"e{hp}{kb}{j}")
                    nc.tensor.matmul(
                        pe[:],
                        lhsT=kdT_sb[64 * j:64 * j + 64, hp, kb * 128:(kb + 1) * 128],
                        rhs=qdT_sb[64 * j:64 * j + 64, hp, :],
                        start=True,
                        stop=True,
                    )
                    nc.scalar.activation(
                        p_tiles[hp][:, kb, j, :], pe[:], EXP, bias=ebias[:]
                    )

        # ---- K-proj B-phase interleaved with even-k (kb=0) energy pairs ----
        kgroups = [
            psA.tile([128, 128], F32, tag="A", name=f"kg{hp}") for hp in range(HP)
        ]
        eq = [(hp, 0) for hp in range(HP)]
        for c in range(NCH):
            for hp in range(HP):
                nc.tensor.matmul(
                    kgroups[hp][:],
                    lhsT=xc(k_t, NCH + c)[:, hp * 128:(hp + 1) * 128],
                    rhs=dctT_sb[:, c, 128:256],
                    start=(c == 0),
                    stop=(c == NCH - 1),
                )
            if eq:
                emit_energy_pair(*eq.pop(0))
        while eq:
            emit_energy_pair(*eq.pop(0))
        for hp in range(HP):
            nc.vector.tensor_copy(kdT_sb[:, hp, 128:256], kgroups[hp][:])

        # ---- phase-B partial-sum tiles: 6 heads share a PSUM bank ----------
        # pc[mb][hg] covers heads hg*6..hg*6+5 of m-block mb.
        pc_tiles = {}

        def pc_of(h, mb):
            hg = h // 6
            key = (mb, hg)
            if key not in pc_tiles:
                pool = psA if mb == 0 else psE
                tag = "A" if mb == 0 else "E"
                pc_tiles[key] = pool.tile(
                    [128, 6, VW], F32, tag=tag, name=f"pc{mb}{hg}"
                )
            return pc_tiles[key][:, h % 6, :]

        def phase_b_mm(h, mb, kb):
            # One accumulation group per PSUM bank: start=True zeroes the
            # whole 2KB zero-region, so only the tile's FIRST matmul starts
            # (the other slices' first writes land on pending-zero bytes and
            # overwrite); only the tile's LAST matmul stops.
            hp, j = h // 2, h % 2
            nc.tensor.matmul(
                pc_of(h, mb),
                lhsT=p_tiles[hp][:, kb, j, mb * 128:(mb + 1) * 128],
                rhs=vd_sb[:, kb, h, :],
                start=(kb == 0 and h % 6 == 0),
                stop=(kb == 1 and h % 6 == 5),
            )

        # ---- V-proj phase A (even parity) + odd-k energy pairs -------------
        def evict_v(groups, par):
            # vd_sb[:, par, h, 0:D] for h in fs*6..fs*6+5, split 3-head-wise
            for fs in range(FSPLIT):
                src = groups[fs][:].rearrange("p (h x) -> p h x", x=D)
                for half in range(2):
                    hs = slice(half * 3, half * 3 + 3)
                    nc.vector.tensor_copy(
                        vd_sb[:, par, fs * 6 + half * 3:fs * 6 + half * 3 + 3, 0:D],
                        src[:, hs, :],
                    )

        vgA = [
            psA.tile([128, FW], F32, tag="A", name=f"vgA{fs}")
            for fs in range(FSPLIT)
        ]
        eq = [(hp, 1) for hp in range(HP)]
        for c in range(NCH):
            for fs in range(FSPLIT):
                nc.tensor.matmul(
                    vgA[fs][:],
                    lhsT=dctT_sb[:, c, 0:128],
                    rhs=xc(v_t, c)[:, fs * FW:(fs + 1) * FW],
                    start=(c == 0),
                    stop=(c == NCH - 1),
                )
            if eq:
                emit_energy_pair(*eq.pop(0))
        while eq:
            emit_energy_pair(*eq.pop(0))
        evict_v(vgA, 0)

        # ---- V-proj phase B (odd parity) + phase-B kb0 pre-accumulation ----
        vgB = [
            psA.tile([128, FW], F32, tag="A", name=f"vgB{fs}")
            for fs in range(FSPLIT)
        ]
        kb0q = [(h, mb) for mb in range(MB) for h in range(H)] if PACK_PC else []
        for c in range(NCH):
            for fs in range(FSPLIT):
                nc.tensor.matmul(
                    vgB[fs][:],
                    lhsT=dctT_sb[:, c, 128:256],
                    rhs=xc(v_t, NCH + c)[:, fs * FW:(fs + 1) * FW],
                    start=(c == 0),
                    stop=(c == NCH - 1),
                )
            for _ in range(3):
                if kb0q:
                    phase_b_mm(*kb0q.pop(0), 0)
        while kb0q:
            phase_b_mm(*kb0q.pop(0), 0)
        evict_v(vgB, 1)

        # ---- phase-B kb1 + batched normalize -------------------------------
        if PACK_PC:
            for mb in range(MB):
                for h in range(H):
                    phase_b_mm(h, mb, 1)
            for (mb, hg), pct in sorted(pc_tiles.items()):
                rs = rbuf.tile([128, 6], F32, tag="r", name=f"r{mb}{hg}")
                nc.vector.reciprocal(rs[:], pct[:, :, D])
                for i in range(6):
                    h = hg * 6 + i
                    dst = ctx_sb[:, mb, h * D:(h + 1) * D]
                    if i % 2 == 0:
                        nc.vector.tensor_scalar_mul(
                            dst, pct[:, i, 0:D], rs[:, i:i + 1]
                        )
                    else:
                        nc.scalar.activation(
                            dst, pct[:, i, 0:D], COPY, scale=rs[:, i:i + 1]
                        )
        else:
            for h in range(H):
                hp, j = h // 2, h % 2
                for mb in range(MB):
                    pc = psA.tile([128, VW], F32, tag="A", name=f"c{h}{mb}")
                    for kb in range(MB):
                        nc.tensor.matmul(
                            pc[:],
                            lhsT=p_tiles[hp][:, kb, j, mb * 128:(mb + 1) * 128],
                            rhs=vd_sb[:, kb, h, :],
                            start=(kb == 0),
                            stop=(kb == MB - 1),
                        )
                    rs = rbuf.tile([128, 1], F32, tag="r", name=f"r{h}{mb}")
                    nc.vector.reciprocal(rs[:], pc[:, D:VW])
                    dst = ctx_sb[:, mb, h * D:(h + 1) * D]
                    if h % 2 == 0:
                        nc.vector.tensor_scalar_mul(dst, pc[:, 0:D], rs[:])
                    else:
                        nc.scalar.activation(dst, pc[:, 0:D], COPY, scale=rs[:])
        # negated odd-k ctx for the reconstructed upper output half
        nc.vector.tensor_scalar_mul(ctxn_sb[:, 0:FW], ctx_sb[:, 1, 0:FW], -1.0)
        nc.vector.tensor_scalar_mul(ctxn_sb[:, FW:HD], ctx_sb[:, 1, FW:HD], -1.0)

        # ---- inverse DCT: full-width row-blocks, out piece every 2 blocks --
        # half 0: x[0:1024] = even + odd contributions.
        # half 1: y[j] = x[N-1-j] = even - odd (host un-reverses rows 1024:,
        # so we accumulate with the negated odd-block ctx).
        for half in range(2):
            for pi in range(NB // PCH):
                ost = ostage.tile(
                    [128, PCH, HD], BF, tag=f"o{pi % 4}", name=f"o{half}{pi}"
                )
                for nbi in range(PCH):
                    nb = pi * PCH + nbi
                    px = [
                        psA.tile([128, FW], F32, tag="A", name=f"x{half}{nb}{fs}")
                        for fs in range(FSPLIT)
                    ]
                    for fs in range(FSPLIT):
                        nc.tensor.matmul(
                            px[fs][:],
                            lhsT=dct_sb[:, 0, nb * 128:(nb + 1) * 128],
                            rhs=ctx_sb[:, 0, fs * FW:(fs + 1) * FW],
                            start=True,
                            stop=False,
                        )
                    for fs in range(FSPLIT):
                        odd_rhs = (
                            ctx_sb[:, 1, fs * FW:(fs + 1) * FW] if half == 0
                            else ctxn_sb[:, fs * FW:(fs + 1) * FW]
                        )
                        nc.tensor.matmul(
                            px[fs][:],
                            lhsT=dct_sb[:, 1, nb * 128:(nb + 1) * 128],
                            rhs=odd_rhs,
                            start=False,
                            stop=True,
                        )
                    for fs in range(FSPLIT):
                        dst = ost[:, nbi, fs * FW:(fs + 1) * FW]
                        if (nb + fs) % 2 == 0:
                            nc.vector.tensor_copy(dst, px[fs][:])
                        else:
                            nc.scalar.activation(dst, px[fs][:], COPY)
                eng = nc.sync
                eng.dma_start(
                    out_r[:, half * NB + pi * PCH:half * NB + (pi + 1) * PCH, :],
                    ost[:],
                )

    nc.compile()
    return nc


def prep_in_maps(Q, K, V, mask, Q_dct):
    Q, K, V = np.asarray(Q), np.asarray(K), np.asarray(V)
    mask, Q_dct = np.asarray(mask), np.asarray(Q_dct)
    scale = np.float32(1.0 / np.sqrt(np.sqrt(np.float32(D))))
    m4 = mask.astype(np.float32)[:, None, :, None]        # [B,1,N,1]

    def fold(x):  # [B,N,HD] -> [A; B] along N
        lo, hi = x[:, :NH, :], x[:, NH:, :][:, ::-1, :]
        return np.concatenate([lo + hi, lo - hi], axis=1)

    qs = fold((Q.astype(np.float32) * scale).transpose(0, 2, 1, 3).reshape(B, N, HD))
    ks = fold((K.astype(np.float32) * scale * m4).transpose(0, 2, 1, 3).reshape(B, N, HD))
    vs = fold((V.astype(np.float32) * m4).transpose(0, 2, 1, 3).reshape(B, N, HD))
    qs = np.ascontiguousarray(qs).astype(_BF16)
    ks = np.ascontiguousarray(ks).astype(_BF16)
    vs = np.ascontiguousarray(vs).astype(_BF16)

    dct_f = Q_dct.astype(np.float32)
    perm = np.concatenate([np.arange(0, M, 2), np.arange(1, M, 2)])
    dct_p = dct_f[perm]                            # rows reordered [even|odd]
    dct = np.ascontiguousarray(dct_p[:, :NH]).astype(_BF16)     # [M, NH]
    dctT = np.ascontiguousarray(dct_p[:, :NH].T).astype(_BF16)  # [NH, M]
    return [
        {"q": qs[b], "k": ks[b], "v": vs[b], "dctT": dctT, "dct": dct}
        for b in range(B)
    ]


def run(Q, K, V, mask, Q_dct, trace=False):
    from concourse.bass_utils import run_bass_kernel_spmd

    if "nc" not in _CACHE:
        _CACHE["nc"] = build_nc()
    nc = _CACHE["nc"]
    in_maps = prep_in_maps(Q, K, V, mask, Q_dct)
    res = run_bass_kernel_spmd(nc, in_maps, core_ids=list(range(B)), trace=trace)
    outs = np.stack(
        [res.results[i]["out"].astype(np.float32) for i in range(B)]
    )  # [B, N, HD]; rows NH: hold y[j] = x[N-1-j] -> un-reverse
    outs[:, NH:, :] = outs[:, NH:, :][:, ::-1, :]
    x = outs.reshape(B, N, H, D).transpose(0, 2, 1, 3)
    return np.ascontiguousarray(x, dtype=np.float32), res


def kernel(Q, K, V, mask, Q_dct):
    x, _ = run(Q, K, V, mask, Q_dct, trace=False)
    return x


# revision 11
# speedup vs baseline: 1.0479x; 1.0479x over previous
"""DCT-attention Trainium2 kernel (8 NeuronCores, data-parallel over batch).

Reference math (per b, h):
    Qd = dct @ (Q*s);  Kd = dct @ (K*s*mask);  Vd = dct @ (V*mask)   # [M,D]
    E  = Qd @ Kd^T;  P = softmax(E, axis=-1);  ctx = P @ Vd          # [M,D]
    x  = dct^T @ ctx                                                 # [N,D]
with B,H,N,D = 8,12,2048,64, M = 256, s = D**-0.25.

Sharding: batch b -> core b (8 cores). Host folds scale into Q/K and mask into
K/V, transposes to [N, H*D], bf16-casts; matmuls run bf16 -> fp32 PSUM; output
returns bf16 and is cast to f32 on the host.

DCT parity symmetry: dct[k, N-1-i] = (-1)^k dct[k, i].  The host uploads X
folded as [A; B] with A = X[:N/2] + reverse(X[N/2:]), B = X[:N/2] -
reverse(X[N/2:]), and the M axis reordered to [even k | odd k]; every
projection then contracts over N/2 instead of N.

Schedule per core (input DMA on the sync HWDGE queue; PE chases arrivals):
  DMA:  dctT | q (2+2+4+4+4 chunks) | k | v (4-chunk quarters) | dct
  PE:   Q-proj A/B, K-proj A (head-pair-major, 2 rotating PSUM banks) ->
        K-proj B + energy pairs (both k-parities per head-pair; each pair
        is two row-tiled 64-contraction matmuls into SEPARATE banks of a
        [128,2,512] tile, one 512-wide exp per pair) ->
        V-proj A -> V-proj B + phase-B kb0 pre-accumulation (24 open
        accumulations packed 6-heads-per-bank) -> kb1 + batched
        normalize -> inverse-DCT sweep (full-width rows, out piece DMA
        every 2 row-blocks).
"""

import numpy as np
import ml_dtypes

B, H, N, D = 8, 12, 2048, 64
M = 256
HD = H * D          # 768
NH = N // 2         # 1024 folded length
NCH = NH // 128     # 8 folded chunks per parity phase
MB = M // 128       # 2 m-blocks (even ks | odd ks)
HP = H // 2         # 6 head-pairs
FSPLIT = 2          # HD split for <=512-wide psum
FW = HD // FSPLIT   # 384
VW = D + 1          # 65: Vd columns + ones column
NB = NH // 128      # 8 output row-blocks per half
PCH = 2             # row-blocks per out piece

_BF16 = ml_dtypes.bfloat16
_CACHE = {}

# q is split finer at the head of the stream so the PE starts sooner.
Q_PIECES = [2, 2, 4, 4, 4]
KV_PIECES = [4, 4, 4, 4]


def build_nc():
    import concourse.bacc as bacc
    import concourse.mybir as mybir
    import concourse.tile as tile
    from contextlib import ExitStack

    BF = mybir.dt.bfloat16
    F32 = mybir.dt.float32
    EXP = mybir.ActivationFunctionType.Exp
    COPY = mybir.ActivationFunctionType.Copy

    nc = bacc.Bacc()
    q_d = nc.declare_dram_parameter("q", [N, HD], BF, isOutput=False)
    k_d = nc.declare_dram_parameter("k", [N, HD], BF, isOutput=False)
    v_d = nc.declare_dram_parameter("v", [N, HD], BF, isOutput=False)
    dctT_d = nc.declare_dram_parameter("dctT", [NH, M], BF, isOutput=False)
    dct_d = nc.declare_dram_parameter("dct", [M, NH], BF, isOutput=False)
    out_d = nc.declare_dram_parameter("out", [N, HD], BF, isOutput=True)

    q_r = q_d.ap().rearrange("(c p) f -> p c f", p=128)
    k_r = k_d.ap().rearrange("(c p) f -> p c f", p=128)
    v_r = v_d.ap().rearrange("(c p) f -> p c f", p=128)
    dctT_r = dctT_d.ap().rearrange("(c p) m -> p c m", p=128)
    dct_r = dct_d.ap().rearrange("(b p) n -> p b n", p=128)
    out_r = out_d.ap().rearrange("(c p) f -> p c f", p=128)

    with ExitStack() as ctx:
        tc = ctx.enter_context(tile.TileContext(nc))
        consts = ctx.enter_context(tc.tile_pool(name="consts", bufs=1))
        xin = ctx.enter_context(tc.tile_pool(name="xin", bufs=1))
        proj = ctx.enter_context(tc.tile_pool(name="proj", bufs=1))
        pbuf = ctx.enter_context(tc.tile_pool(name="pbuf", bufs=1))
        rbuf = ctx.enter_context(tc.tile_pool(name="rbuf", bufs=4))
        ostage = ctx.enter_context(tc.tile_pool(name="ostage", bufs=4))
        psA = ctx.enter_context(tc.tile_pool(name="psA", bufs=4, space="PSUM"))
        psE = ctx.enter_context(tc.tile_pool(name="psE", bufs=2, space="PSUM"))

        # ---- DMA stream (sync queue; order == consumption order) ----
        dctT_sb = consts.tile([128, NCH, M], BF)       # [n'-part, chunk, m]
        nc.sync.dma_start(dctT_sb[:], dctT_r)

        def stream(name, src_r, pieces):
            chunk_map = []
            c0 = 0
            for pi, nch in enumerate(pieces):
                t = xin.tile([128, nch, HD], BF, tag=f"{name}{pi}")
                nc.sync.dma_start(t[:], src_r[:, c0:c0 + nch, :])
                for j in range(nch):
                    chunk_map.append((t, j))
                c0 += nch
            return chunk_map

        q_t = stream("q", q_r, Q_PIECES)   # chunks 0..7 = A-fold, 8..15 = B
        k_t = stream("k", k_r, KV_PIECES)
        v_t = stream("v", v_r, KV_PIECES)

        dct_sb = consts.tile([128, MB, NH], BF)        # [m-part, m-block, n']
        nc.sync.dma_start(dct_sb[:], dct_r)

        # ---- persistent intermediates ----
        qdT_sb = proj.tile([128, HP, M], BF, tag="qdT")   # [2-head d, pair, m]
        kdT_sb = proj.tile([128, HP, M], BF, tag="kdT")
        vd_sb = proj.tile([128, MB, H, VW], BF, tag="vd")  # [k-part, kb, h, d+1]
        ctx_sb = proj.tile([128, MB, HD], BF, tag="ctx")   # [m-part, mb, h*d]
        ctxn_sb = proj.tile([128, HD], BF, tag="ctxn")     # -ctx odd block
        nc.vector.memset(vd_sb[:, :, :, D:VW], 1.0)
        ebias = consts.tile([128, 1], F32)
        nc.vector.memset(ebias[:], -4.0)

        def xc(cm, c):  # folded chunk c (0..15), [128, HD]
            t, j = cm[c]
            return t[:, j, :]

        # ---- Q/K projections: head-pair-major, 2 rotating PSUM banks ------
        def proj_group(cm, dst_sb, par, hp):
            g = psA.tile([128, 128], F32, tag="A", name=f"pg{par}{hp}")
            for c in range(NCH):
                nc.tensor.matmul(
                    g[:],
                    lhsT=xc(cm, par * NCH + c)[:, hp * 128:(hp + 1) * 128],
                    rhs=dctT_sb[:, c, par * 128:(par + 1) * 128],
                    start=(c == 0),
                    stop=(c == NCH - 1),
                )
            nc.vector.tensor_copy(
                dst_sb[:, hp, par * 128:(par + 1) * 128], g[:]
            )

        for hp in range(HP):
            proj_group(q_t, qdT_sb, 0, hp)
        for hp in range(HP):
            proj_group(q_t, qdT_sb, 1, hp)
        for hp in range(HP):
            proj_group(k_t, kdT_sb, 0, hp)

        # ---- energy pair: 2 heads of one head-pair; separate banks --------
        p_tiles = [None] * HP   # [128, MB, 2, M] bf16 per head-pair

        def emit_energy_pair(hp, kb):
            if p_tiles[hp] is None:
                p_tiles[hp] = pbuf.tile(
                    [128, MB, 2, M], BF, tag=f"p{hp}", name=f"p{hp}"
                )
            # [128, 2, 512] fp32 = 2 banks; head j's energy lands in bank j
            # (row-tiled 64-contraction matmuls run concurrently and must
            # not share a PSUM bank).
            pe = psE.tile([128, 2, 2 * M], F32, tag="E", name=f"e{hp}{kb}")
            for j in range(2):
                nc.tensor.matmul(
                    pe[:, j, 0:M],
                    lhsT=kdT_sb[64 * j:64 * j + 64, hp, kb * 128:(kb + 1) * 128],
                    rhs=qdT_sb[64 * j:64 * j + 64, hp, :],
                    start=True,
                    stop=True,
                )
            # P^T[k-block, m] = exp(E^T - 4); the -4 cancels in the
            # normalization and guards exp overflow for outlier logits.
            nc.scalar.activation(
                p_tiles[hp][:, kb, :, :], pe[:, :, 0:M], EXP, bias=ebias[:]
            )

        # ---- K-proj B + all energy pairs ----------------------------------
        for hp in range(HP):
            proj_group(k_t, kdT_sb, 1, hp)
            emit_energy_pair(hp, 0)
            emit_energy_pair(hp, 1)

        # ---- phase-B partial sums: 6 heads share a PSUM bank --------------
        pc_tiles = {}

        def pc_of(h, mb):
            hg = h // 6
            key = (mb, hg)
            if key not in pc_tiles:
                pool, tag = (psA, "A") if mb == 0 else (psE, "E")
                pc_tiles[key] = pool.tile(
                    [128, 6, VW], F32, tag=tag, name=f"pc{mb}{hg}"
                )
            return pc_tiles[key][:, h % 6, :]

        def phase_b_mm(h, mb, kb):
            # One accumulation group per PSUM bank: start=True zeroes the
            # whole 2KB zero-region, so only the tile's FIRST matmul starts;
            # the other slices' first writes land on pending-zero bytes and
            # overwrite.  Only the tile's LAST matmul stops.
            hp, j = h // 2, h % 2
            nc.tensor.matmul(
                pc_of(h, mb),
                lhsT=p_tiles[hp][:, kb, j, mb * 128:(mb + 1) * 128],
                rhs=vd_sb[:, kb, h, :],
                start=(kb == 0 and h % 6 == 0),
                stop=(kb == 1 and h % 6 == 5),
            )

        # ---- V-proj phase A (even parity) ---------------------------------
        def evict_v(groups, par):
            for fs in range(FSPLIT):
                src = groups[fs][:].rearrange("p (h x) -> p h x", x=D)
                for half in range(2):
                    h0 = fs * 6 + half * 3
                    nc.vector.tensor_copy(
                        vd_sb[:, par, h0:h0 + 3, 0:D],
                        src[:, half * 3:half * 3 + 3, :],
                    )

        vgA = [
            psA.tile([128, FW], F32, tag="A", name=f"vgA{fs}")
            for fs in range(FSPLIT)
        ]
        for c in range(NCH):
            for fs in range(FSPLIT):
                nc.tensor.matmul(
                    vgA[fs][:],
                    lhsT=dctT_sb[:, c, 0:128],
                    rhs=xc(v_t, c)[:, fs * FW:(fs + 1) * FW],
                    start=(c == 0),
                    stop=(c == NCH - 1),
                )
        evict_v(vgA, 0)

        # ---- V-proj phase B (odd parity) + phase-B kb0 pre-accumulation ---
        vgB = [
            psA.tile([128, FW], F32, tag="A", name=f"vgB{fs}")
            for fs in range(FSPLIT)
        ]
        kb0q = [(h, mb) for mb in range(MB) for h in range(H)]
        for c in range(NCH):
            for fs in range(FSPLIT):
                nc.tensor.matmul(
                    vgB[fs][:],
                    lhsT=dctT_sb[:, c, 128:256],
                    rhs=xc(v_t, NCH + c)[:, fs * FW:(fs + 1) * FW],
                    start=(c == 0),
                    stop=(c == NCH - 1),
                )
            for _ in range(3):
                if kb0q:
                    phase_b_mm(*kb0q.pop(0), 0)
        while kb0q:
            phase_b_mm(*kb0q.pop(0), 0)
        evict_v(vgB, 1)

        # ---- phase-B kb1 + batched normalize ------------------------------
        for mb in range(MB):
            for h in range(H):
                phase_b_mm(h, mb, 1)
        for (mb, hg), pct in sorted(pc_tiles.items()):
            rs = rbuf.tile([128, 6], F32, tag="r", name=f"r{mb}{hg}")
            nc.vector.reciprocal(rs[:], pct[:, :, D])
            for i in range(6):
                h = hg * 6 + i
                dst = ctx_sb[:, mb, h * D:(h + 1) * D]
                if i % 2 == 0:
                    nc.vector.tensor_scalar_mul(
                        dst, pct[:, i, 0:D], rs[:, i:i + 1]
                    )
                else:
                    nc.scalar.activation(
                        dst, pct[:, i, 0:D], COPY, scale=rs[:, i:i + 1]
                    )
        # negated odd-k ctx for the reconstructed upper output half
        nc.vector.tensor_scalar_mul(ctxn_sb[:, 0:FW], ctx_sb[:, 1, 0:FW], -1.0)
        nc.vector.tensor_scalar_mul(ctxn_sb[:, FW:HD], ctx_sb[:, 1, FW:HD], -1.0)

        # ---- inverse DCT: full-width row-blocks, out piece every 2 blocks -
        # half 0: x[0:1024] = even + odd contributions.
        # half 1: y[j] = x[N-1-j] = even - odd (host un-reverses rows 1024:,
        # so we accumulate with the negated odd-block ctx).
        for half in range(2):
            for pi in range(NB // PCH):
                ost = ostage.tile(
                    [128, PCH, HD], BF, tag=f"o{pi % 4}", name=f"o{half}{pi}"
                )
                for nbi in range(PCH):
                    nb = pi * PCH + nbi
                    px = [
                        psA.tile([128, FW], F32, tag="A", name=f"x{half}{nb}{fs}")
                        for fs in range(FSPLIT)
                    ]
                    for fs in range(FSPLIT):
                        nc.tensor.matmul(
                            px[fs][:],
                            lhsT=dct_sb[:, 0, nb * 128:(nb + 1) * 128],
                            rhs=ctx_sb[:, 0, fs * FW:(fs + 1) * FW],
                            start=True,
                            stop=False,
                        )
                    for fs in range(FSPLIT):
                        odd_rhs = (
                            ctx_sb[:, 1, fs * FW:(fs + 1) * FW] if half == 0
                            else ctxn_sb[:, fs * FW:(fs + 1) * FW]
                        )
                        nc.tensor.matmul(
                            px[fs][:],
                            lhsT=dct_sb[:, 1, nb * 128:(nb + 1) * 128],
                            rhs=odd_rhs,
                            start=False,
                            stop=True,
                        )
                    for fs in range(FSPLIT):
                        dst = ost[:, nbi, fs * FW:(fs + 1) * FW]
                        if (nb + fs) % 2 == 0:
                            nc.vector.tensor_copy(dst, px[fs][:])
                        else:
                            nc.scalar.activation(dst, px[fs][:], COPY)
                nc.sync.dma_start(
                    out_r[:, half * NB + pi * PCH:half * NB + (pi + 1) * PCH, :],
                    ost[:],
                )

    nc.compile()
    return nc


def prep_in_maps(Q, K, V, mask, Q_dct):
    Q, K, V = np.asarray(Q), np.asarray(K), np.asarray(V)
    mask, Q_dct = np.asarray(mask), np.asarray(Q_dct)
    scale = np.float32(1.0 / np.sqrt(np.sqrt(np.float32(D))))
    m4 = mask.astype(np.float32)[:, None, :, None]        # [B,1,N,1]

    def fold(x):  # [B,N,HD] -> [A; B] along N
        lo, hi = x[:, :NH, :], x[:, NH:, :][:, ::-1, :]
        return np.concatenate([lo + hi, lo - hi], axis=1)

    qs = fold((Q.astype(np.float32) * scale).transpose(0, 2, 1, 3).reshape(B, N, HD))
    ks = fold((K.astype(np.float32) * scale * m4).transpose(0, 2, 1, 3).reshape(B, N, HD))
    vs = fold((V.astype(np.float32) * m4).transpose(0, 2, 1, 3).reshape(B, N, HD))
    qs = np.ascontiguousarray(qs).astype(_BF16)
    ks = np.ascontiguousarray(ks).astype(_BF16)
    vs = np.ascontiguousarray(vs).astype(_BF16)

    dct_f = Q_dct.astype(np.float32)
    perm = np.concatenate([np.arange(0, M, 2), np.arange(1, M, 2)])
    dct_p = dct_f[perm]                            # rows reordered [even|odd]
    dct = np.ascontiguousarray(dct_p[:, :NH]).astype(_BF16)     # [M, NH]
    dctT = np.ascontiguousarray(dct_p[:, :NH].T).astype(_BF16)  # [NH, M]
    return [
        {"q": qs[b], "k": ks[b], "v": vs[b], "dctT": dctT, "dct": dct}
        for b in range(B)
    ]


def run(Q, K, V, mask, Q_dct, trace=False):
    from concourse.bass_utils import run_bass_kernel_spmd

    if "nc" not in _CACHE:
        _CACHE["nc"] = build_nc()
    nc = _CACHE["nc"]
    in_maps = prep_in_maps(Q, K, V, mask, Q_dct)
    res = run_bass_kernel_spmd(nc, in_maps, core_ids=list(range(B)), trace=trace)
    outs = np.stack(
        [res.results[i]["out"].astype(np.float32) for i in range(B)]
    )  # [B, N, HD]; rows NH: hold y[j] = x[N-1-j] -> un-reverse
    outs[:, NH:, :] = outs[:, NH:, :][:, ::-1, :]
    x = outs.reshape(B, N, H, D).transpose(0, 2, 1, 3)
    return np.ascontiguousarray(x, dtype=np.float32), res


def kernel(Q, K, V, mask, Q_dct):
    x, _ = run(Q, K, V, mask, Q_dct, trace=False)
    return x
